# revision 13
# baseline (speedup 1.0000x reference)
"""Trainium2 Bass kernel for ConvertMomentsTEtoParamsTEGaussLayer.

Math (per sample b, all 128x128 f32):
    A = chol[b]; C = covTE[b]
    phi = A^T @ C @ A
    cholTE[b] = -A @ (mask * phi),  mask = tril(ones) - 0.5*eye
Output: (muTE, cholTE) -- muTE passes through unchanged.

Mapping onto the PE primitive mm(L, R) = L^T @ R (contracts partition dims):
    X   = mm(C, A)  = C^T A          (PSUM -> SBUF via ACT copy)
    phi = mm(X, A)  = A^T C A        (PSUM)
    R   = negmask * phi              (DVE, PSUM->SBUF)
    out = mm(A^T, R) = A @ R         (A^T provided pre-transposed from host)

Sharding: pure data parallel, batch 4096 -> 512 samples on each of 8 cores.
Host pre-swizzles each input into chunk-major layout [nchunk, 128, G*128]
so every chunk DMA is one fully contiguous transfer.
"""

import numpy as np

B = 4096
N = 128
NCORES = 8
BPC = B // NCORES  # 512 samples per core
G = 16             # samples per chunk (1 MiB per chunk per stream)
NCHUNK = BPC // G  # 32
IN_BUFS = 5

_CACHE = {}


def _build_nc(nchunk, g, in_bufs=IN_BUFS):
    from contextlib import ExitStack

    import concourse.bass as bass
    import concourse.tile as tile
    from concourse import bacc, mybir

    dt = mybir.dt.float32
    nc = bacc.Bacc(
        "TRN2", target_bir_lowering=False, debug=False, num_devices=NCORES
    )

    gw = g * N  # free-dim width of one chunk tile
    cov_d = nc.dram_tensor("cov_r", [nchunk, N, gw], dt, kind="ExternalInput").ap()
    a_d = nc.dram_tensor("chol_r", [nchunk, N, gw], dt, kind="ExternalInput").ap()
    at_d = nc.dram_tensor("cholT_r", [nchunk, N, gw], dt, kind="ExternalInput").ap()
    mask_d = nc.dram_tensor("negmask", [N, N], dt, kind="ExternalInput").ap()
    out_d = nc.dram_tensor("out_r", [nchunk, N, gw], dt, kind="ExternalOutput").ap()

    S = nchunk * g

    with tile.TileContext(nc) as tc, ExitStack() as ctx:
        cpool = ctx.enter_context(tc.tile_pool(name="cpool", bufs=in_bufs))
        apool = ctx.enter_context(tc.tile_pool(name="apool", bufs=in_bufs))
        atpool = ctx.enter_context(tc.tile_pool(name="atpool", bufs=in_bufs))
        opool = ctx.enter_context(tc.tile_pool(name="opool", bufs=2))
        xsb_pool = ctx.enter_context(tc.tile_pool(name="xsb", bufs=4))
        rsb_pool = ctx.enter_context(tc.tile_pool(name="rsb", bufs=4))
        const_pool = ctx.enter_context(tc.tile_pool(name="const", bufs=1))
        xps_pool = ctx.enter_context(tc.tile_pool(name="xps", bufs=2, space="PSUM"))
        pps_pool = ctx.enter_context(tc.tile_pool(name="pps", bufs=3, space="PSUM"))
        ops_pool = ctx.enter_context(tc.tile_pool(name="ops", bufs=3, space="PSUM"))

        # mask on the ACT HWDGE ring: off the load-critical sync ring, and
        # arrives well before the first DVE use (~2 samples in)
        mask_sb = const_pool.tile([N, N], dt)
        nc.scalar.dma_start(mask_sb[:], mask_d[:])

        ctile, atile, attile, otile = {}, {}, {}, {}
        xsb, rsb, pps = {}, {}, {}

        def load_chunk(c, slices=1):
            # slices>1: split each stream's DMA so the first samples of the
            # chunk become ready early (used for chunk 0 to cut the head
            # stall before the first matmul). C/A slices go first: mm3's
            # A^T stream is only needed two pipeline stages later.
            ctile[c] = cpool.tile([N, gw], dt, name=f"ct{c}", tag="ct")
            atile[c] = apool.tile([N, gw], dt, name=f"at{c}", tag="at")
            attile[c] = atpool.tile([N, gw], dt, name=f"att{c}", tag="att")
            w = gw // slices
            for i in range(slices):
                sl = slice(i * w, (i + 1) * w)
                nc.sync.dma_start(ctile[c][:, sl], cov_d[c][:, sl])
                nc.sync.dma_start(atile[c][:, sl], a_d[c][:, sl])
            for i in range(slices):
                sl = slice(i * w, (i + 1) * w)
                nc.sync.dma_start(attile[c][:, sl], at_d[c][:, sl])

        def col(gg):
            return slice(gg * N, (gg + 1) * N)

        load_chunk(0, slices=4)
        for c0 in range(1, min(in_bufs - 1, nchunk)):
            load_chunk(c0)
        # 3-stage software pipeline over samples: consecutive PE matmuls are
        # independent so the PE never stalls on the ACT/DVE drains in between.
        for s in range(S + 2):
            if s < S:
                c, gg = divmod(s, g)
                if gg == 0:
                    nxt = c + in_bufs - 1
                    if nxt < nchunk:
                        load_chunk(nxt)
                    otile[c] = opool.tile([N, gw], dt, name=f"ot{c}", tag="ot")
                # stage 1: X = C^T A
                xps = xps_pool.tile([N, N], dt)
                nc.tensor.matmul(
                    xps[:], lhsT=ctile[c][:, col(gg)], rhs=atile[c][:, col(gg)],
                    start=True, stop=True,
                )
                xsb[s] = xsb_pool.tile([N, N], dt, name=f"xsb{s}", tag="xsb")
                nc.scalar.copy(xsb[s][:], xps[:])
            t = s - 1
            if 0 <= t < S:
                c, gg = divmod(t, g)
                # stage 2: phi = X^T A = A^T C A, then R = negmask * phi
                pps[t] = pps_pool.tile([N, N], dt, name=f"pps{t}", tag="pps")
                nc.tensor.matmul(
                    pps[t][:], lhsT=xsb[t][:], rhs=atile[c][:, col(gg)],
                    start=True, stop=True,
                )
                rsb[t] = rsb_pool.tile([N, N], dt, name=f"rsb{t}", tag="rsb")
                nc.vector.tensor_mul(rsb[t][:], pps[t][:], mask_sb[:])
                del xsb[t], pps[t]
            u = s - 2
            if 0 <= u < S:
                c, gg = divmod(u, g)
                # stage 3: out = (A^T)^T R = A R
                ops = ops_pool.tile([N, N], dt)
                nc.tensor.matmul(
                    ops[:], lhsT=attile[c][:, col(gg)], rhs=rsb[u][:],
                    start=True, stop=True,
                )
                nc.vector.tensor_copy(otile[c][:, col(gg)], ops[:])
                del rsb[u]
                # stores on the ACT HWDGE ring (separate queue from loads),
                # issued per few samples so the tail store is short
                sp = min(8, g)
                if (gg + 1) % sp == 0:
                    st = slice((gg + 1 - sp) * N, (gg + 1) * N)
                    nc.scalar.dma_start(out_d[c][:, st], otile[c][:, st])
                if gg == g - 1:
                    del ctile[c], atile[c], attile[c], otile[c]

    nc.compile()
    return nc


def _get_nc(nchunk=NCHUNK, g=G):
    key = (nchunk, g)
    if key not in _CACHE:
        _CACHE[key] = _build_nc(nchunk, g)
    return _CACHE[key]


def _swizzle(x, ncores, nchunk, g):
    # [B, N, N] -> per-core chunk-major [ncores, nchunk, N, g*N]
    b = x.shape[0]
    assert b == ncores * nchunk * g
    return (
        np.ascontiguousarray(
            x.reshape(ncores, nchunk, g, N, N).transpose(0, 1, 3, 2, 4)
        ).reshape(ncores, nchunk, N, g * N)
    )


def _unswizzle(y, nchunk, g):
    # [nchunk, N, g*N] -> [nchunk*g, N, N]
    return (
        y.reshape(nchunk, N, g, N).transpose(0, 2, 1, 3).reshape(nchunk * g, N, N)
    )


def _negmask():
    m = np.tril(np.ones((N, N), dtype=np.float32)) - 0.5 * np.eye(N, dtype=np.float32)
    return np.ascontiguousarray(-m)


def _make_in_maps(covTE, chol, ncores, nchunk, g):
    cholT = np.ascontiguousarray(chol.transpose(0, 2, 1))
    cov_r = _swizzle(covTE, ncores, nchunk, g)
    a_r = _swizzle(chol, ncores, nchunk, g)
    at_r = _swizzle(cholT, ncores, nchunk, g)
    mask = _negmask()
    return [
        {
            "cov_r": cov_r[i],
            "chol_r": a_r[i],
            "cholT_r": at_r[i],
            "negmask": mask,
        }
        for i in range(ncores)
    ]


def kernel(muTE, covTE, chol):
    from concourse.bass_utils import run_bass_kernel_spmd

    muTE = np.asarray(muTE, dtype=np.float32)
    covTE = np.asarray(covTE, dtype=np.float32)
    chol = np.asarray(chol, dtype=np.float32)

    nc = _get_nc()
    in_maps = _make_in_maps(covTE, chol, NCORES, NCHUNK, G)
    res = run_bass_kernel_spmd(nc, in_maps, list(range(NCORES)))
    cholTE = np.concatenate(
        [_unswizzle(res.results[i]["out_r"], NCHUNK, G) for i in range(NCORES)],
        axis=0,
    )
    return (muTE, cholTE)


# revision 14
# speedup vs baseline: 1.0922x; 1.0922x over previous
"""Trainium2 Bass kernel for ConvertMomentsTEtoParamsTEGaussLayer.

Math (per sample b, all 128x128 f32):
    A = chol[b]; C = covTE[b]
    phi = A^T @ C @ A
    cholTE[b] = -A @ (mask * phi),  mask = tril(ones) - 0.5*eye
Output: (muTE, cholTE) -- muTE passes through unchanged.

Mapping onto the PE primitive mm(L, R) = L^T @ R (contracts partition dims):
    X   = mm(C, A)  = C^T A          (PSUM -> SBUF via ACT copy)
    phi = mm(X, A)  = A^T C A        (PSUM)
    R   = negmask * phi              (DVE, PSUM->SBUF)
    out = mm(A^T, R) = A @ R         (A^T provided pre-transposed from host)

Sharding: pure data parallel, batch 4096 -> 512 samples on each of 8 cores.
Host pre-swizzles each input into chunk-major layout [nchunk, 128, G*128]
so every chunk DMA is one fully contiguous transfer.
"""

import numpy as np

B = 4096
N = 128
NCORES = 8
BPC = B // NCORES  # 512 samples per core
G = 32             # samples per chunk (2 MiB per chunk per stream)
NCHUNK = BPC // G  # 16
IN_BUFS = 3

_CACHE = {}


def _build_nc(nchunk, g, in_bufs=IN_BUFS):
    from contextlib import ExitStack

    import concourse.bass as bass
    import concourse.tile as tile
    from concourse import bacc, mybir

    dt = mybir.dt.float32
    nc = bacc.Bacc(
        "TRN2", target_bir_lowering=False, debug=False, num_devices=NCORES
    )

    gw = g * N  # free-dim width of one chunk tile
    cov_d = nc.dram_tensor("cov_r", [nchunk, N, gw], dt, kind="ExternalInput").ap()
    a_d = nc.dram_tensor("chol_r", [nchunk, N, gw], dt, kind="ExternalInput").ap()
    at_d = nc.dram_tensor("cholT_r", [nchunk, N, gw], dt, kind="ExternalInput").ap()
    mask_d = nc.dram_tensor("negmask", [N, N], dt, kind="ExternalInput").ap()
    out_d = nc.dram_tensor("out_r", [nchunk, N, gw], dt, kind="ExternalOutput").ap()

    S = nchunk * g

    with tile.TileContext(nc) as tc, ExitStack() as ctx:
        cpool = ctx.enter_context(tc.tile_pool(name="cpool", bufs=in_bufs))
        apool = ctx.enter_context(tc.tile_pool(name="apool", bufs=in_bufs))
        atpool = ctx.enter_context(tc.tile_pool(name="atpool", bufs=in_bufs))
        opool = ctx.enter_context(tc.tile_pool(name="opool", bufs=2))
        xsb_pool = ctx.enter_context(tc.tile_pool(name="xsb", bufs=4))
        rsb_pool = ctx.enter_context(tc.tile_pool(name="rsb", bufs=4))
        const_pool = ctx.enter_context(tc.tile_pool(name="const", bufs=1))
        xps_pool = ctx.enter_context(tc.tile_pool(name="xps", bufs=2, space="PSUM"))
        pps_pool = ctx.enter_context(tc.tile_pool(name="pps", bufs=3, space="PSUM"))
        ops_pool = ctx.enter_context(tc.tile_pool(name="ops", bufs=3, space="PSUM"))

        mask_sb = const_pool.tile([N, N], dt)
        nc.sync.dma_start(mask_sb[:], mask_d[:])

        ctile, atile, attile, otile = {}, {}, {}, {}
        xsb, rsb, pps = {}, {}, {}

        def load_chunk(c, slices=1):
            # slices>1: split each stream's DMA so the first samples of the
            # chunk become ready early (used for chunk 0 to cut the head
            # stall before the first matmul). C/A slices go first: mm3's
            # A^T stream is only needed two pipeline stages later.
            ctile[c] = cpool.tile([N, gw], dt, name=f"ct{c}", tag="ct")
            atile[c] = apool.tile([N, gw], dt, name=f"at{c}", tag="at")
            attile[c] = atpool.tile([N, gw], dt, name=f"att{c}", tag="att")
            w = gw // slices
            for i in range(slices):
                sl = slice(i * w, (i + 1) * w)
                nc.sync.dma_start(ctile[c][:, sl], cov_d[c][:, sl])
                nc.sync.dma_start(atile[c][:, sl], a_d[c][:, sl])
                nc.sync.dma_start(attile[c][:, sl], at_d[c][:, sl])

        def col(gg):
            return slice(gg * N, (gg + 1) * N)

        load_chunk(0, slices=4)
        for c0 in range(1, min(in_bufs - 1, nchunk)):
            load_chunk(c0)
        # 3-stage software pipeline over samples: consecutive PE matmuls are
        # independent so the PE never stalls on the ACT/DVE drains in between.
        for s in range(S + 2):
            if s < S:
                c, gg = divmod(s, g)
                if gg == 0:
                    nxt = c + in_bufs - 1
                    if nxt < nchunk:
                        load_chunk(nxt)
                    otile[c] = opool.tile([N, gw], dt, name=f"ot{c}", tag="ot")
                # stage 1: X = C^T A
                xps = xps_pool.tile([N, N], dt)
                nc.tensor.matmul(
                    xps[:], lhsT=ctile[c][:, col(gg)], rhs=atile[c][:, col(gg)],
                    start=True, stop=True,
                )
                xsb[s] = xsb_pool.tile([N, N], dt, name=f"xsb{s}", tag="xsb")
                nc.scalar.copy(xsb[s][:], xps[:])
            t = s - 1
            if 0 <= t < S:
                c, gg = divmod(t, g)
                # stage 2: phi = X^T A = A^T C A, then R = negmask * phi
                pps[t] = pps_pool.tile([N, N], dt, name=f"pps{t}", tag="pps")
                nc.tensor.matmul(
                    pps[t][:], lhsT=xsb[t][:], rhs=atile[c][:, col(gg)],
                    start=True, stop=True,
                )
                rsb[t] = rsb_pool.tile([N, N], dt, name=f"rsb{t}", tag="rsb")
                nc.vector.tensor_mul(rsb[t][:], pps[t][:], mask_sb[:])
                del xsb[t], pps[t]
            u = s - 2
            if 0 <= u < S:
                c, gg = divmod(u, g)
                # stage 3: out = (A^T)^T R = A R
                ops = ops_pool.tile([N, N], dt)
                nc.tensor.matmul(
                    ops[:], lhsT=attile[c][:, col(gg)], rhs=rsb[u][:],
                    start=True, stop=True,
                )
                nc.vector.tensor_copy(otile[c][:, col(gg)], ops[:])
                del rsb[u]
                # stores on the ACT HWDGE ring (separate queue from loads),
                # issued per few samples so the tail store is short
                sp = min(8, g)
                if (gg + 1) % sp == 0:
                    st = slice((gg + 1 - sp) * N, (gg + 1) * N)
                    nc.scalar.dma_start(out_d[c][:, st], otile[c][:, st])
                if gg == g - 1:
                    del ctile[c], atile[c], attile[c], otile[c]

    nc.compile()
    return nc


def _get_nc(nchunk=NCHUNK, g=G):
    key = (nchunk, g)
    if key not in _CACHE:
        _CACHE[key] = _build_nc(nchunk, g)
    return _CACHE[key]


def _swizzle(x, ncores, nchunk, g):
    # [B, N, N] -> per-core chunk-major [ncores, nchunk, N, g*N]
    b = x.shape[0]
    assert b == ncores * nchunk * g
    return (
        np.ascontiguousarray(
            x.reshape(ncores, nchunk, g, N, N).transpose(0, 1, 3, 2, 4)
        ).reshape(ncores, nchunk, N, g * N)
    )


def _unswizzle(y, nchunk, g):
    # [nchunk, N, g*N] -> [nchunk*g, N, N]
    return (
        y.reshape(nchunk, N, g, N).transpose(0, 2, 1, 3).reshape(nchunk * g, N, N)
    )


def _negmask():
    m = np.tril(np.ones((N, N), dtype=np.float32)) - 0.5 * np.eye(N, dtype=np.float32)
    return np.ascontiguousarray(-m)


def _make_in_maps(covTE, chol, ncores, nchunk, g):
    cholT = np.ascontiguousarray(chol.transpose(0, 2, 1))
    cov_r = _swizzle(covTE, ncores, nchunk, g)
    a_r = _swizzle(chol, ncores, nchunk, g)
    at_r = _swizzle(cholT, ncores, nchunk, g)
    mask = _negmask()
    return [
        {
            "cov_r": cov_r[i],
            "chol_r": a_r[i],
            "cholT_r": at_r[i],
            "negmask": mask,
        }
        for i in range(ncores)
    ]


def kernel(muTE, covTE, chol):
    from concourse.bass_utils import run_bass_kernel_spmd

    muTE = np.asarray(muTE, dtype=np.float32)
    covTE = np.asarray(covTE, dtype=np.float32)
    chol = np.asarray(chol, dtype=np.float32)

    nc = _get_nc()
    in_maps = _make_in_maps(covTE, chol, NCORES, NCHUNK, G)
    res = run_bass_kernel_spmd(nc, in_maps, list(range(NCORES)))
    cholTE = np.concatenate(
        [_unswizzle(res.results[i]["out_r"], NCHUNK, G) for i in range(NCORES)],
        axis=0,
    )
    return (muTE, cholTE)
